# revision 1
# baseline (speedup 1.0000x reference)
"""GCN connectivity kernel for 8 Trainium2 NeuronCores.

Pipeline (per the reference):
    h1 = relu(Ahat @ (x @ W1) + b1)
    h2 = relu(Ahat @ (h1 @ W2) + b2)
    out = tanh(h2 @ Wfc + bfc);  result = (out + out.T) / 2

with Ahat[d, s] = dinv[d] * dinv[s] * cnt[d, s], cnt = edge counts incl.
self-loops, deg = in-degree of the loop-augmented dst list.

Distribution: nodes (and output rows) are sharded 1024/core.

Message passing is dense matmuls against the per-core adjacency-count slice,
stored as EXACT small integers in fp8e4 and kept resident in SBUF
(cnt^T slice is the moving operand; the fp16 node-feature table is the
stationary operand; psum accumulates [64 feat x 512 dst] over 64 k-tiles).
The dinv normalization is applied around the relu on the DVE using
host-precomputed broadcast tiles:
    t1 = relu(dinv^2 * S1 + dinv*b1)   (feeds table2 = t1 @ W2)
    t2 = relu(dinv * S2 + b2)          (= h2, feature-major)
using relu positive-homogeneity to fold the next layer's src-side dinv.

Small activation tables are exchanged with three AllGather collectives.

The final fc + tanh + symmetrize is computed without any transposes:
    result[i, j] = sigmoid(2 z[i, j]) - sigmoid(-2 z[j, i])
both z row-blocks and (negated) z^T row-blocks are K=65 matmuls of
feature-major factors (bias via an appended ones/bias row); the negated
z^T block shares one packed [128 x 4096] PSUM window with z so a single
Sigmoid(scale=2) activation covers both, then one fp16 DVE subtract and
one DMA store per [128 x 2048] output tile.
"""

import numpy as np

import concourse.bass as bass
import concourse.mybir as mybir
import concourse.tile as tile
from concourse import bacc
from concourse import bass_utils

FP8 = mybir.dt.float8e4
FP16 = mybir.dt.float16
FP32 = mybir.dt.float32
AF = mybir.ActivationFunctionType
ALU = mybir.AluOpType

N, E, F, H, C = 8192, 524288, 512, 64, 8


def build_program(n=N, f=F, h=H, c=C, js=1024, at_dt=FP8):
    """Build the (SPMD, identical-on-every-core) bass program."""
    ns = n // c        # nodes per core
    kt = n // 128      # src k-tiles in message passing
    gw = min(512, ns)   # dst-group width (matmul out is capped at one PSUM bank)
    g = ns // gw       # dst groups per core
    nt = ns // 128     # 128-row node tiles per core
    fb = f // 128      # k-tiles of the input-feature dim
    nj = n // js       # output column supers
    jc = js // 512     # 512-wide matmul chunks per super

    nc = bacc.Bacc(
        "TRN2",
        target_bir_lowering=False,
        debug=False,
        num_devices=c,
    )

    at = nc.dram_tensor("at", [n, ns], at_dt, kind="ExternalInput").ap()
    xt = nc.dram_tensor("xt", [f, ns], FP16, kind="ExternalInput").ap()
    w1 = nc.dram_tensor("w1", [f, h], FP16, kind="ExternalInput").ap()
    w2 = nc.dram_tensor("w2", [h, h], FP16, kind="ExternalInput").ap()
    wfca = nc.dram_tensor("wfca", [h + 1, n], FP16, kind="ExternalInput").ap()
    # NEGATED Wfc[:, rows] | bfc[rows] so z^T psums hold -z^T and share the
    # z sigmoid's scale=+2
    wfcin = nc.dram_tensor("wfcin", [h + 1, ns], FP16, kind="ExternalInput").ap()
    dv1 = nc.dram_tensor("dv1", [h, ns], FP32, kind="ExternalInput").ap()
    dv2 = nc.dram_tensor("dv2", [h, ns], FP32, kind="ExternalInput").ap()
    btx1 = nc.dram_tensor("btx1", [h, ns], FP32, kind="ExternalInput").ap()
    b2d = nc.dram_tensor("b2d", [h, 1], FP32, kind="ExternalInput").ap()
    out = nc.dram_tensor("out", [ns, n], FP16, kind="ExternalOutput").ap()

    groups = [list(range(c))]

    with tile.TileContext(nc, num_cores=c) as tc:
        with (
            tc.tile_pool(name="const", bufs=1) as constp,
            tc.tile_pool(name="dram", bufs=1, space="DRAM") as dramp,
        ):
            # ---------- persistent SBUF tensors ----------
            at_g = [
                constp.tile(
                    [128, kt * gw], at_dt, name=f"atg{gi}", tag=f"atg{gi}"
                )
                for gi in range(g)
            ]
            xt_sb = constp.tile([128, fb * ns], FP16)
            w1_sb = constp.tile([128, fb * h], FP16)
            w2_sb = constp.tile([h, h], FP16)
            wfca_sb = constp.tile([h + 1, n], FP16)
            wfcin_sb = constp.tile([h + 1, ns], FP16)
            table_sb = constp.tile([128, kt * h], FP16)
            t1_sb = constp.tile([h, ns], FP16)
            t2loc_sb = constp.tile([h + 1, ns], FP16)
            h2t_sb = constp.tile([h + 1, n], FP16)
            zeros_sb = constp.tile([h, gw], FP16)
            dv1_sb = constp.tile([h, ns], FP32)
            dv2_sb = constp.tile([h, ns], FP32)
            btx1_sb = constp.tile([h, ns], FP32)
            b2_sb = constp.tile([h, 1], FP32)

            nc.gpsimd.memset(zeros_sb[:], 0.0)
            nc.gpsimd.memset(t2loc_sb[h : h + 1, :], 1.0)
            nc.gpsimd.memset(h2t_sb[h : h + 1, :], 1.0)

            # critical-path loads first (xt -> p1 -> AllGather gates MP1);
            # the big adjacency load goes on the SWDGE queue so it streams
            # in parallel with the HWDGE input loads.
            nc.sync.dma_start(
                xt_sb[:].rearrange("p (kb m) -> p kb m", kb=fb),
                xt.rearrange("(kb p) m -> p kb m", p=128),
            )
            nc.sync.dma_start(
                w1_sb[:].rearrange("p (kb q) -> p kb q", kb=fb),
                w1.rearrange("(kb p) q -> p kb q", p=128),
            )
            nc.sync.dma_start(w2_sb[:], w2[:])
            nc.sync.dma_start(dv1_sb[:], dv1[:])
            nc.sync.dma_start(dv2_sb[:], dv2[:])
            nc.sync.dma_start(btx1_sb[:], btx1[:])
            nc.sync.dma_start(b2_sb[:], b2d[:])
            # resident adjacency, split per dst group so group 0's matmuls
            # can start at the half-way point: at_g[gi][p, k*gw + m] =
            # at[k*128 + p, gi*gw + m]
            for gi in range(g):
                nc.sync.dma_start(
                    at_g[gi][:].rearrange("p (k m) -> p k m", k=kt),
                    at[:, gi * gw : (gi + 1) * gw].rearrange(
                        "(k p) m -> p k m", p=128
                    ),
                )

            # ---------- DRAM bounce buffers for the collectives ----------
            # AG1/AG2 shards are bounced pre-swizzled as [128p, nt*h] so the
            # gathered result is already in table layout: core cc's block is
            # table_sb[:, cc*nt*h : (cc+1)*nt*h] (its nodes are exactly the
            # contiguous k-range [cc*nt, (cc+1)*nt)).
            ag1_in = dramp.tile([128, nt * h], FP16)
            ag1_out = dramp.tile([c * 128, nt * h], FP16)
            ag2_in = dramp.tile([128, nt * h], FP16)
            ag2_out = dramp.tile([c * 128, nt * h], FP16)
            ag3_in = dramp.tile([h, ns], FP16)
            ag3_out = dramp.tile([c, h, ns], FP16)
            pst_sb = constp.tile([128, nt * h], FP16)
            # warm the ACT Sigmoid table set off the critical path: this
            # scrap write lands in pst_sb, which phase 0 fully overwrites
            # before its first reader
            nc.scalar.activation(
                pst_sb[0:1, 0:8], zeros_sb[0:1, 0:8], AF.Sigmoid, scale=2.0
            )

            def load_table(ag_out):
                for cc in range(c):
                    nc.sync.dma_start(
                        table_sb[:, cc * nt * h : (cc + 1) * nt * h],
                        ag_out[cc * 128 : (cc + 1) * 128, :],
                    )

            with (
                tc.tile_pool(name="tmp", bufs=2) as tmpp,
                tc.tile_pool(name="mpps", bufs=2, space="PSUM") as mpps,
            ):
                # ------ phase 0: p1' = (dinv*x) @ W1 (own rows) ------
                for it in range(nt):
                    ps = mpps.tile([128, h], FP32, tag="p0")
                    for kb in range(fb):
                        nc.tensor.matmul(
                            ps[:],
                            lhsT=xt_sb[
                                :, kb * ns + it * 128 : kb * ns + (it + 1) * 128
                            ],
                            rhs=w1_sb[:, kb * h : (kb + 1) * h],
                            start=(kb == 0),
                            stop=(kb == fb - 1),
                        )
                    nc.vector.tensor_copy(
                        pst_sb[:, it * h : (it + 1) * h], ps[:]
                    )
                nc.gpsimd.dma_start(ag1_in[:], pst_sb[:])

                nc.gpsimd.collective_compute(
                    "AllGather",
                    ALU.bypass,
                    replica_groups=groups,
                    ins=[ag1_in[:].opt()],
                    outs=[ag1_out[:].opt()],
                )
                load_table(ag1_out)

                # ------ dense message-passing matmuls for one dst group ------
                def mp_group(gi):
                    ps = mpps.tile([h, gw], FP32, tag="mp")
                    for k in range(kt):
                        nc.tensor.matmul(
                            ps[:],
                            lhsT=table_sb[:, k * h : (k + 1) * h],
                            rhs=at_g[gi][:, k * gw : (k + 1) * gw],
                            start=(k == 0),
                            stop=(k == kt - 1),
                        )
                    return ps

                # ------ layer 1:  t1 = relu(dinv^2*S1 + dinv*b1) ------
                for gi in range(g):
                    sl = slice(gi * gw, (gi + 1) * gw)
                    ps = mp_group(gi)
                    u = tmpp.tile([h, gw], FP32, tag="u")
                    nc.vector.tensor_tensor(
                        out=u[:], in0=ps[:], in1=dv2_sb[:, sl], op=ALU.mult
                    )
                    nc.vector.tensor_tensor(
                        out=u[:], in0=u[:], in1=btx1_sb[:, sl], op=ALU.add
                    )
                    nc.vector.tensor_scalar_max(t1_sb[:, sl], u[:], 0.0)

                # table2 = t1 @ W2, node-major shard, then gather
                for it in range(nt):
                    ps = mpps.tile([128, h], FP32, tag="p0")
                    nc.tensor.matmul(
                        ps[:],
                        lhsT=t1_sb[:, it * 128 : (it + 1) * 128],
                        rhs=w2_sb[:],
                        start=True,
                        stop=True,
                    )
                    nc.vector.tensor_copy(
                        pst_sb[:, it * h : (it + 1) * h], ps[:]
                    )
                nc.gpsimd.dma_start(ag2_in[:], pst_sb[:])

                nc.gpsimd.collective_compute(
                    "AllGather",
                    ALU.bypass,
                    replica_groups=groups,
                    ins=[ag2_in[:].opt()],
                    outs=[ag2_out[:].opt()],
                )
                load_table(ag2_out)
                # fc-only weights: loaded here so they never sit ahead of the
                # activation-table loads in the sync DMA FIFO
                nc.sync.dma_start(wfca_sb[:], wfca[:])
                nc.sync.dma_start(wfcin_sb[:], wfcin[:])

                # ------ layer 2:  t2 = h2 = relu(dinv*S2 + b2) ------
                for gi in range(g):
                    sl = slice(gi * gw, (gi + 1) * gw)
                    ps = mp_group(gi)
                    u = tmpp.tile([h, gw], FP32, tag="u")
                    nc.vector.tensor_tensor(
                        out=u[:], in0=ps[:], in1=dv1_sb[:, sl], op=ALU.mult
                    )
                    nc.vector.scalar_tensor_tensor(
                        out=t2loc_sb[0:h, sl],
                        in0=u[:],
                        scalar=b2_sb[:],
                        in1=zeros_sb[:],
                        op0=ALU.add,
                        op1=ALU.max,
                    )

                nc.gpsimd.dma_start(ag3_in[:], t2loc_sb[0:h, :])
                nc.gpsimd.collective_compute(
                    "AllGather",
                    ALU.bypass,
                    replica_groups=groups,
                    ins=[ag3_in[:].opt()],
                    outs=[ag3_out[:].opt()],
                )
                # h2t_sb[q, cc*ns + m] = ag3_out[cc, q, m]
                for cc in range(c):
                    nc.sync.dma_start(
                        h2t_sb[0:h, cc * ns : (cc + 1) * ns],
                        ag3_out[cc, :, :],
                    )

            # ---------- fc + tanh + symmetrize ----------
            with (
                tc.tile_pool(name="fcps", bufs=2, space="PSUM") as fcps,
                tc.tile_pool(name="fcsb", bufs=2) as fcsb,
            ):
                for it in range(nt):
                    isl = slice(it * 128, (it + 1) * 128)
                    for j in range(nj):
                        pzz = fcps.tile([128, 2 * js], FP32, tag="pzz")
                        for q in range(jc):
                            sl = slice(j * js + q * 512, j * js + (q + 1) * 512)
                            qsl = slice(q * 512, (q + 1) * 512)
                            nqsl = slice(js + q * 512, js + (q + 1) * 512)
                            nc.tensor.matmul(
                                pzz[:, qsl],
                                lhsT=t2loc_sb[:, isl],
                                rhs=wfca_sb[:, sl],
                                start=True,
                                stop=True,
                            )
                            nc.tensor.matmul(
                                pzz[:, nqsl],
                                lhsT=wfcin_sb[:, isl],
                                rhs=h2t_sb[:, sl],
                                start=True,
                                stop=True,
                            )
                        s12 = fcsb.tile([128, 2 * js], FP16, tag="s12")
                        ot = fcsb.tile([128, js], FP16, tag="ot")
                        nc.scalar.activation(s12[:], pzz[:], AF.Sigmoid, scale=2.0)
                        nc.vector.tensor_tensor(
                            out=ot[:],
                            in0=s12[:, 0:js],
                            in1=s12[:, js : 2 * js],
                            op=ALU.subtract,
                        )
                        nc.sync.dma_start(
                            out[isl, j * js : (j + 1) * js],
                            ot[:],
                        )

    return nc


def host_prep(x, edge_index, W1, b1, W2, b2, Wfc, bfc, n, c):
    """Build the per-core input maps (all graph prep happens here)."""
    ns = n // c
    x = np.asarray(x, np.float32)
    ei = np.asarray(edge_index).astype(np.int64)
    W1 = np.asarray(W1, np.float32)
    W2 = np.asarray(W2, np.float32)
    Wfc = np.asarray(Wfc, np.float32)
    b1 = np.asarray(b1, np.float32)
    b2 = np.asarray(b2, np.float32)
    bfc = np.asarray(bfc, np.float32)

    loops = np.arange(n, dtype=np.int64)
    s_all = np.concatenate([ei[0], loops])
    d_all = np.concatenate([ei[1], loops])
    deg = np.bincount(d_all, minlength=n).astype(np.float32)
    dinv = np.where(deg > 0, deg ** -0.5, 0.0).astype(np.float32)

    # exact small-integer edge counts (fp8e4 represents 0..15 exactly)
    cnt = np.zeros((n, n), np.float32)
    np.add.at(cnt, (d_all, s_all), 1.0)

    import ml_dtypes

    fp8 = ml_dtypes.float8_e4m3

    wfca = np.concatenate([Wfc, bfc[None, :]], axis=0).astype(np.float16)
    xs = x * dinv[:, None]  # fold src-side dinv of layer 1 into x

    in_maps = []
    for ci in range(c):
        rows = slice(ci * ns, (ci + 1) * ns)
        dloc = dinv[rows]
        in_maps.append(
            {
                "at": np.ascontiguousarray(cnt[rows, :].T).astype(fp8),
                "xt": np.ascontiguousarray(xs[rows, :].T).astype(np.float16),
                "w1": W1.astype(np.float16),
                "w2": W2.astype(np.float16),
                "wfca": wfca,
                "wfcin": np.ascontiguousarray(-wfca[:, rows]),
                "dv1": np.repeat(dloc[None, :], W1.shape[1], axis=0).astype(
                    np.float32
                ),
                "dv2": np.repeat((dloc * dloc)[None, :], W1.shape[1], axis=0)
                .astype(np.float32),
                "btx1": np.ascontiguousarray(
                    b1[:, None] * dloc[None, :]
                ).astype(np.float32),
                "b2d": b2.reshape(-1, 1).astype(np.float32),
            }
        )
    return in_maps


_cached = {}


def _get_program(key):
    if key not in _cached:
        n, f, h, c = key
        nc = build_program(n=n, f=f, h=h, c=c)
        nc.finalize()
        _cached[key] = nc
    return _cached[key]


def run(inputs, n=N, f=F, h=H, c=C, trace=False):
    nc = _get_program((n, f, h, c))
    in_maps = host_prep(
        inputs["x"], inputs["edge_index"], inputs["W1"], inputs["b1"],
        inputs["W2"], inputs["b2"], inputs["Wfc"], inputs["bfc"], n, c,
    )
    res = bass_utils.run_bass_kernel_spmd(
        nc, in_maps, core_ids=list(range(c)), trace=trace
    )
    parts = [res.results[ci]["out"].astype(np.float32) for ci in range(c)]
    return np.concatenate(parts, axis=0), res


def kernel(**inputs) -> np.ndarray:
    out, _ = run(inputs)
    return out

